# revision 39
# baseline (speedup 1.0000x reference)
"""Causal self-attention (B=2, S=2048, D=1024, H=16) on 8 TRN2 NeuronCores.

Sharding: tensor-parallel over heads (2 heads/core) for qkv+attention,
then AllToAll to token-parallel (512 tokens/core) for the output projection.

Schedule (per core, SPMD):
  1. qkv^T projection with stationary-weight reuse: for each of V,K,Q and
     each 128-deep contraction chunk, one LDWEIGHTS serves 8 token-chunk
     matmuls accumulating into 8 PSUM banks (bf16 MMs, fp32 PSUM).
     x^T streamed from DRAM in 1MB chunks; evictions (+bias, ->bf16) are
     split across vector and scalar engines.
  2. V^T -> V via DMA-engine xbar transposes (off the PE), ones column
     appended so AV also produces the softmax denominator.
  3. Attention, flat-pipelined across (hl, b, qc): S^T score tiles
     (keys on partitions) in 2-tile PSUM groups, one grouped exp on
     scalar (scale=1/8 fused), causal = block skipping + triangle mask
     multiply; AV accumulates y^T+denom in PSUM. Softmax normalization
     is done sender-side: reciprocal of the denom row, PE-broadcast,
     fused into the f32->bf16 cast of y.
  4. TWO AllToAlls (one per local head, bf16, ~0.5MB/rank each): the
     first fires while the second head's attention still computes; the
     second overlaps the first half of the output projection.
  5. Output projection for my 512 tokens with PSUM accumulation split
     into per-piece contraction chunks (W_proj rows host-permuted so
     each AllToAll piece forms whole 128-deep chunks); bias via a
     fp32r ones-row matmul; fp32 out.
Host gathers the 8 token-slices and reshapes.
"""

import numpy as np
from contextlib import ExitStack

import concourse.bass as bass
import concourse.bacc as bacc
import concourse.tile as tile
from concourse import mybir
from concourse.bass_utils import run_bass_kernel_spmd
from concourse.masks import make_identity

B, S, D = 2, 2048, 1024
H, HD = 16, 64
NCORE = 8
HPC = H // NCORE          # heads per core = 2
CW = HPC * HD             # channels per core = 128
T = B * S                 # 4096 tokens
TPC = T // NCORE          # 512 tokens per core (proj phase)
TCH = 512                 # token chunk for qkv projection
NT = T // TCH             # 8
QCH = 512                 # query chunk
KCH = 128                 # key chunk
NQC = S // QCH            # 4 query chunks per batch
DK = D // 128             # 8 contraction chunks of 128
GT = 2                    # score tiles per exp group (psA slot = [128, GT, 512])
LOOK = 2                  # score-group lookahead over AV

f32 = mybir.dt.float32
f32r = mybir.dt.float32r
bf16 = mybir.dt.bfloat16
AF = mybir.ActivationFunctionType


def _build():
    nc = bacc.Bacc(None, target_bir_lowering=False, num_devices=NCORE)

    xT = nc.dram_tensor("xT", [D, T], bf16, kind="ExternalInput")
    wq = nc.dram_tensor("wq", [D, CW], bf16, kind="ExternalInput")
    wk = nc.dram_tensor("wk", [D, CW], bf16, kind="ExternalInput")
    wv = nc.dram_tensor("wv", [D, CW], bf16, kind="ExternalInput")
    bqkv = nc.dram_tensor("bqkv", [3, CW], f32, kind="ExternalInput")
    wp = nc.dram_tensor("wp", [D, D], bf16, kind="ExternalInput")  # rows permuted
    bp = nc.dram_tensor("bp", [1, D], f32, kind="ExternalInput")
    out = nc.dram_tensor("out", [TPC, D], f32, kind="ExternalOutput")

    with ExitStack() as ctx:
        tc = ctx.enter_context(tile.TileContext(nc))
        const = ctx.enter_context(tc.tile_pool(name="const", bufs=1))
        dram = ctx.enter_context(tc.tile_pool(name="dram", bufs=1, space="DRAM"))
        wpool = ctx.enter_context(tc.tile_pool(name="wpool", bufs=1))
        xpool = ctx.enter_context(tc.tile_pool(name="xpool", bufs=1))
        qkvt_pool = ctx.enter_context(tc.tile_pool(name="qkvt", bufs=1))
        vpool = ctx.enter_context(tc.tile_pool(name="vpool", bufs=4))
        ptpool = ctx.enter_context(tc.tile_pool(name="ptp", bufs=3))
        npool = ctx.enter_context(tc.tile_pool(name="npool", bufs=2))
        rpool = ctx.enter_context(tc.tile_pool(name="rpool", bufs=1))
        opool = ctx.enter_context(tc.tile_pool(name="opool", bufs=3))
        psA = ctx.enter_context(tc.tile_pool(name="psA", bufs=3, space="PSUM"))
        psB = ctx.enter_context(tc.tile_pool(name="psB", bufs=2, space="PSUM"))

        # ---- constants (no DMA) ----
        identity = const.tile([128, 128], bf16)
        make_identity(nc, identity[:])
        # mask[k, q] = 1.0 if k <= q else 0.0  (keep lower-left in S^T layout)
        mask = const.tile([128, 128], bf16)
        nc.gpsimd.memset(mask[:], 0.0)
        nc.gpsimd.affine_select(
            out=mask[:], in_=mask[:],
            compare_op=mybir.AluOpType.is_ge,  # iota(k-q-1) >= 0 (k>q) -> keep 0; else fill 1
            fill=1.0, base=-1, pattern=[[-1, 128]], channel_multiplier=1,
        )
        ones_f32 = const.tile([128, 128], f32)
        nc.vector.memset(ones_f32[:], 1.0)
        ones_bf = const.tile([128, 64], bf16)
        nc.vector.memset(ones_bf[:], 1.0)
        ones_row = const.tile([1, 128], f32r)
        nc.vector.tensor_copy(ones_row[:], ones_f32[0:1, :])
        ones_fr = const.tile([128, 64], f32r)
        nc.vector.tensor_copy(ones_fr[:], ones_f32[:, 0:64])

        # ---- input DMAs: order matters (single sync issue queue) ----
        wv_sb = wpool.tile([128, DK, CW], bf16)
        wk_sb = wpool.tile([128, DK, CW], bf16)
        wq_sb = wpool.tile([128, DK, CW], bf16)
        nc.sync.dma_start(wv_sb[:], wv[:].rearrange("(c p) m -> p c m", p=128))
        xt = xpool.tile([128, DK, T], bf16)
        xr = xT[:].rearrange("(c p) t -> p c t", p=128)
        HT = T // 2
        nc.sync.dma_start(xt[:, 0, 0:HT], xr[:, 0, 0:HT])
        nc.sync.dma_start(xt[:, 1, 0:HT], xr[:, 1, 0:HT])
        bias_sb = const.tile([128, 3], f32)
        nc.sync.dma_start(bias_sb[:], bqkv[:].rearrange("g p -> p g"))
        nc.sync.dma_start(wk_sb[:], wk[:].rearrange("(c p) m -> p c m", p=128))
        nc.sync.dma_start(wq_sb[:], wq[:].rearrange("(c p) m -> p c m", p=128))
        for c in range(2, DK):
            nc.sync.dma_start(xt[:, c, 0:HT], xr[:, c, 0:HT])
        bp_sb = const.tile([1, D], f32r)
        nc.sync.dma_start(bp_sb[:], bp[:].bitcast(f32r))
        for c in range(DK):
            nc.sync.dma_start(xt[:, c, HT:T], xr[:, c, HT:T])
        wp_sb = wpool.tile([128, DK, D], bf16)
        nc.sync.dma_start(wp_sb[:], wp[:].rearrange("(c p) m -> p c m", p=128))

        # ---- A2A buffers: one piece per local head, normalized bf16 ----
        send_t = [dram.tile([NCORE, HD, TPC], bf16, name=f"send{hl}")
                  for hl in range(HPC)]
        recv_t = [dram.tile([NCORE, HD, TPC], bf16, name=f"recv{hl}")
                  for hl in range(HPC)]

        # ---- phase 1: qkv^T projection, stationary-weight reuse; two passes
        # over token halves so the PE chases only half the x^T DMA ----
        qT = qkvt_pool.tile([128, T], bf16)
        kT = qkvt_pool.tile([128, T], bf16)
        vT = qkvt_pool.tile([128, T], bf16)
        vlocs = {}
        for half in range(2):
            for wsb, dst, gi in ((wv_sb, vT, 2), (wk_sb, kT, 1), (wq_sb, qT, 0)):
                accA = [psA.tile([128, GT, TCH], f32, tag="a",
                                 name=f"qacc{half}{gi}{i}") for i in range(2)]

                def acc_t(t4):
                    return accA[t4 // 2][:, t4 % 2, :]

                for c in range(DK):
                    for t4 in range(4):
                        t = half * 4 + t4
                        nc.tensor.matmul(
                            acc_t(t4), lhsT=wsb[:, c, :],
                            rhs=xt[:, c, t * TCH:(t + 1) * TCH],
                            start=(c == 0), stop=(c == DK - 1),
                        )
                for t4 in range(4):
                    t = half * 4 + t4
                    dsl = dst[:, t * TCH:(t + 1) * TCH]
                    if t4 < 2:
                        nc.vector.tensor_scalar_add(dsl, acc_t(t4),
                                                    bias_sb[:, gi:gi + 1])
                    else:
                        nc.scalar.activation(dsl, acc_t(t4), AF.Identity,
                                             bias=bias_sb[:, gi:gi + 1], scale=1.0)
                if gi == 2:
                    # this half's V done: V^T -> V via DMA xbar transpose
                    b = half
                    for hl in range(HPC):
                        r0 = hl * HD
                        vloc = vpool.tile([128, (S // KCH) * (HD + 1)], bf16,
                                          tag="v", name=f"vloc_{b}_{hl}")
                        vlocs[(b, hl)] = vloc
                        vv = vloc[:].rearrange("p (c w) -> p c w", w=HD + 1)
                        nc.vector.tensor_copy(vv[:, :, HD], ones_bf[:, 0:S // KCH])
                        vdat = vpool.tile([128, S // KCH, HD], bf16, tag="vd",
                                          bufs=2, name=f"vdat_{b}_{hl}")
                        nc.sync.dma_start_transpose(
                            vdat[:], vT[r0:r0 + HD, b * S:(b + 1) * S])
                        nc.vector.tensor_copy(vv[:, :, 0:HD], vdat[:])

        # ---- phase 3: attention, flat-pipelined across (hl, b, qc) ----
        def koff_of(qc, kc):
            return max(0, kc * KCH - qc * QCH)

        groups = []
        for hl in range(HPC):
            for b in range(B):
                for qc in range(NQC):
                    kcs_all = list(range(4 * (qc + 1)))
                    for i in range(0, len(kcs_all), GT):
                        groups.append((hl, b, qc, kcs_all[i:i + GT]))

        pts = {}
        ypss = {}
        pending = []  # (emit_cycle, hl, b, qc, ypss_tile, recip_tile)
        state = {"cycle": 0, "hl0_sent": 0, "a2a0": False}
        rsbs = [rpool.tile([128, 4, TPC], bf16, name=f"rsb{hl}")
                for hl in range(HPC)]

        def emit_scores(g):
            hl, b, qc, kcs = g
            r0 = hl * HD
            sps = psA.tile([128, GT, QCH], f32, tag="a", name=f"sps{state['cycle']}")
            for i, kc in enumerate(kcs):
                ko = koff_of(qc, kc)
                nc.tensor.matmul(
                    sps[:, i, ko:QCH],
                    lhsT=kT[r0:r0 + HD, b * S + kc * KCH: b * S + (kc + 1) * KCH],
                    rhs=qT[r0:r0 + HD, b * S + qc * QCH + ko: b * S + (qc + 1) * QCH],
                    start=True, stop=True,
                )
            pt = ptpool.tile([128, GT, QCH], bf16, tag="pt", name=f"pt{state['cycle']}")
            w0 = koff_of(qc, kcs[0])
            nc.scalar.activation(pt[:, :, w0:QCH], sps[:, :, w0:QCH], AF.Exp,
                                 scale=0.125)
            for i, kc in enumerate(kcs):
                if kc * KCH >= qc * QCH:  # triangle tile
                    ko = koff_of(qc, kc)
                    nc.vector.tensor_mul(pt[:, i, ko:ko + KCH],
                                         pt[:, i, ko:ko + KCH], mask[:])
            pts[tuple(g[:3]) + (kcs[0],)] = pt

        def flush_norms():
            while pending and pending[0][0] <= state["cycle"] - 2:
                _, hl, b, qc, yp, rfull = pending.pop(0)
                bcp = psA.tile([HD, QCH], f32, tag="a", name=f"bc{hl}{b}{qc}")
                nc.tensor.matmul(bcp[:], lhsT=ones_fr[64:65, :],
                                 rhs=rfull[64:65, :],
                                 start=True, stop=True)
                bcs = npool.tile([HD, QCH], bf16, tag="c", name=f"bs{hl}{b}{qc}")
                nc.vector.tensor_copy(bcs[:], bcp[:])
                sendb = npool.tile([HD, QCH], bf16, tag="s", name=f"sb{hl}{b}{qc}")
                nc.vector.tensor_mul(sendb[:], yp[0:HD, :], bcs[:])
                nc.sync.dma_start(send_t[hl][b * NQC + qc], sendb[:])
                if hl == 0:
                    state["hl0_sent"] += 1
                    if state["hl0_sent"] == B * NQC and not state["a2a0"]:
                        state["a2a0"] = True
                        nc.gpsimd.collective_compute(
                            "AllToAll", mybir.AluOpType.bypass,
                            replica_groups=[list(range(NCORE))],
                            ins=[send_t[0][:].opt()], outs=[recv_t[0][:].opt()],
                        )
                        nc.sync.dma_start(
                            rsbs[0][:],
                            recv_t[0][:].rearrange("(c two) h t -> (two h) c t",
                                                   two=2))

        def emit_av(g):
            flush_norms()
            hl, b, qc, kcs = g
            key3 = tuple(g[:3])
            if kcs[0] == 0:
                ypss[key3] = psB.tile([HD + 1, QCH], f32, tag="b",
                                      name=f"yps{hl}{b}{qc}")
            yp = ypss[key3]
            vloc = vlocs[(b, hl)]
            pt = pts.pop(key3 + (kcs[0],))
            last_kc = 4 * qc + 3
            for i, kc in enumerate(kcs):
                ko = koff_of(qc, kc)
                nc.tensor.matmul(
                    yp[:, ko:QCH],
                    lhsT=vloc[:, kc * (HD + 1):(kc + 1) * (HD + 1)],
                    rhs=pt[:, i, ko:QCH],
                    start=(kc == 0), stop=(kc == last_kc),
                )
            if kcs[-1] == last_kc:
                # reciprocal of the denominator row, in place on partition 64
                rfull = npool.tile([128, QCH], f32r, tag="r", name=f"rf{hl}{b}{qc}")
                with nc.allow_low_precision(reason="softmax recip in f32r"):
                    nc.vector.reciprocal(rfull[64:65, :], yp[HD:HD + 1, :])
                pending.append((state["cycle"], hl, b, qc, yp, rfull))

        for gi in range(len(groups) + LOOK):
            state["cycle"] = gi
            if gi < len(groups):
                emit_scores(groups[gi])
            if gi - LOOK >= 0:
                emit_av(groups[gi - LOOK])
        state["cycle"] += 10
        flush_norms()

        # ---- phase 4b: second AllToAll ----
        nc.gpsimd.collective_compute(
            "AllToAll", mybir.AluOpType.bypass,
            replica_groups=[list(range(NCORE))],
            ins=[send_t[1][:].opt()], outs=[recv_t[1][:].opt()],
        )

        # ---- phase 5: output projection, incremental over A2A pieces ----
        nc.sync.dma_start(
            rsbs[1][:],
            recv_t[1][:].rearrange("(c two) h t -> (two h) c t", two=2))
        oaccA = [psA.tile([128, 2, TCH], f32, tag="a", name=f"oacc{i}")
                 for i in range(3)]
        oaccB = [psB.tile([128, TCH], f32, tag="b", name=f"oaccb{i}")
                 for i in range(2)]

        def oacc(m, n):
            return oaccA[m][:, n, :] if m < 3 else oaccB[n][:]

        for piece in range(HPC):
            rsb = rsbs[piece]
            for m in range(TPC // 128):
                for ci in range(4):
                    c = piece * 4 + ci
                    for n in range(2):
                        nc.tensor.matmul(
                            oacc(m, n),
                            lhsT=rsb[:, ci, m * 128:(m + 1) * 128],
                            rhs=wp_sb[:, c, n * 512:(n + 1) * 512],
                            start=(c == 0), stop=False,
                        )
        for m in range(TPC // 128):
            for n in range(2):
                nc.tensor.matmul(
                    oacc(m, n), lhsT=ones_row[:],
                    rhs=bp_sb[:, n * 512:(n + 1) * 512],
                    start=False, stop=True,
                )
                osb = opool.tile([128, 512], f32, tag="osb")
                nc.vector.tensor_copy(osb[:], oacc(m, n))
                nc.sync.dma_start(out[m * 128:(m + 1) * 128, n * 512:(n + 1) * 512],
                                  osb[:])

    nc.compile()
    return nc


_NC_CACHE = None


def _get_nc():
    global _NC_CACHE
    if _NC_CACHE is None:
        _NC_CACHE = _build()
    return _NC_CACHE


def _bf16(a):
    import ml_dtypes
    return np.ascontiguousarray(a.astype(ml_dtypes.bfloat16))


def _in_maps(x, W_attn, b_attn, W_proj, b_proj):
    x = np.ascontiguousarray(np.asarray(x, dtype=np.float32))
    W_attn = np.asarray(W_attn, dtype=np.float32)
    b_attn = np.asarray(b_attn, dtype=np.float32)
    W_proj = np.ascontiguousarray(np.asarray(W_proj, dtype=np.float32))
    b_proj = np.asarray(b_proj, dtype=np.float32)

    xT = _bf16(x.reshape(T, D).T)  # [D, T] bf16
    # permute W_proj rows so A2A piece hl supplies whole 128-deep chunks:
    # new row order: for hl: for k (src core): the 64 channels (k, hl)
    perm = np.concatenate([
        np.arange(D).reshape(NCORE, HPC, HD)[:, hl, :].reshape(-1)
        for hl in range(HPC)
    ])
    wp16 = _bf16(W_proj[perm, :])
    bp2 = np.ascontiguousarray(b_proj.reshape(1, D))
    maps = []
    for c in range(NCORE):
        lo = c * CW
        sl_q = slice(lo, lo + CW)
        sl_k = slice(D + lo, D + lo + CW)
        sl_v = slice(2 * D + lo, 2 * D + lo + CW)
        maps.append({
            "xT": xT,
            "wq": _bf16(W_attn[:, sl_q]),
            "wk": _bf16(W_attn[:, sl_k]),
            "wv": _bf16(W_attn[:, sl_v]),
            "bqkv": np.ascontiguousarray(
                np.stack([b_attn[sl_q], b_attn[sl_k], b_attn[sl_v]])),
            "wp": wp16,
            "bp": bp2,
        })
    return maps


def _gather(results):
    outs = [np.asarray(r["out"]) for r in results]
    return np.concatenate(outs, axis=0).reshape(B, S, D)


def kernel(x, W_attn, b_attn, W_proj, b_proj):
    nc = _get_nc()
    maps = _in_maps(x, W_attn, b_attn, W_proj, b_proj)
    res = run_bass_kernel_spmd(nc, maps, core_ids=list(range(NCORE)))
    return _gather(res.results)


def kernel_traced(x, W_attn, b_attn, W_proj, b_proj, **kw):
    """Same as kernel() but with NTFF tracing; returns (out, BassKernelResults)."""
    nc = _get_nc()
    maps = _in_maps(x, W_attn, b_attn, W_proj, b_proj)
    res = run_bass_kernel_spmd(nc, maps, core_ids=list(range(NCORE)), trace=True, **kw)
    return _gather(res.results), res
